# revision 37
# baseline (speedup 1.0000x reference)
"""Trainium2 Bass kernel for nn_BasicBlock (spiking CNN block).

Sharding: data-parallel over batch B across 8 NeuronCores (4 batch x 4
timesteps = 16 images per core); BN batch stats via tiny AllReduce.

Per core:
- conv1: 3x3 taps as TensorEngine matmuls in fp16 hi/lo split arithmetic
  (~fp32 accuracy at fp16 speed). Hi pass: per tap [W1hi;W1hi] x [xhi;xlo]
  (K=128, 9 matmuls). Lo pass tap-paired: a second plane copy holds xhi
  shifted one column, so [W1lo(di,0);W1lo(di,1)] x [xhi;xhi_sh] computes two
  taps per matmul (3 pair + 3 single = 6 matmuls instead of 9).
- BN stats (sum / sum-of-squares) accumulated during PSUM evacuation
  (ScalarE copy w/ accum_out; DVE square pass), all-reduced across cores.
- PLIF scan in "q-space" (conv-output units): BN scale/bias folded into
  per-channel threshold theta / constants, so no per-element BN apply.
- conv2 consumes exact 0/1 spikes in fp16: per tap [W2hi;W2lo] x [s1;s1]
  (K=128) gives both split terms in one matmul.
- Residual + LIF2: fused r = x*rsc2 + y2 on GpSimd, spike emitted as
  Sign(q2 - th2) on the Act engine in fp16; host maps sign>=0 -> 0/1.
"""
import sys
sys.path.insert(0, '/opt/trn_rl_repo')

import numpy as np

T, B, C, H, W = 4, 32, 64, 56, 56
NCORES = 8
BL = B // NCORES            # 4 local batch samples
NIMG = T * BL               # 16 images per core
HP = W + 2                  # 58
PP = HP * HP                # 3364 padded pixels
PIX = H * W                 # 3136
NCH = 7                     # conv chunks per image (8 rows each)
CHW = 8 * W                 # 448
NPAIR = 8                   # image pairs per core
EPS = 1e-5
NG = float((T * B) * PIX)   # 401408
QL = 14 * W                 # LIF quarter-strip length (784)
NQ = 4

_prog_cache = {}
DBG = False
NO_CC = False
PHASES = 3
TRACE = False
LAST_RES = None
LAST_NAMES = None
LAST_EXEC_NS = None


def _build(alpha1, alpha2):
    import concourse.mybir as mybir
    import concourse.tile as tile
    from concourse import bacc

    F32 = mybir.dt.float32
    F16 = mybir.dt.float16
    F8 = mybir.dt.float8e4
    AO = mybir.AluOpType
    AF = mybir.ActivationFunctionType
    AX = mybir.AxisListType
    PM = mybir.MatmulPerfMode

    nc = bacc.Bacc(None, target_bir_lowering=False)
    names = {}

    # conv1 op list: (plane_key, lhs_name, lhs_col, di, dj)
    ops1 = [("pc", "w1a", a * 64, a // 3, a % 3) for a in range(9)] \
        + [("ph", "w1bp", di * 64, di, 0) for di in range(3)] \
        + [("ph", "w1bs", di * 64, di, 2) for di in range(3)]

    with tile.TileContext(nc) as tc:
        with tc.tile_pool(name="dram", bufs=1, space="DRAM") as dram:
            xta = dram.tile([NIMG, 2, 64, PP], F16, kind="ExternalInput")
            xsh = dram.tile([NIMG, 64, PP], F16, kind="ExternalInput")
            xin = dram.tile([NIMG, 64, PIX], F32, kind="ExternalInput")
            w1a = dram.tile([128, 9 * 64], F16, kind="ExternalInput")
            w1bp = dram.tile([128, 3 * 64], F16, kind="ExternalInput")
            w1bs = dram.tile([128, 3 * 64], F16, kind="ExternalInput")
            w2a8 = dram.tile([128, 2 * 9 * 64], F8, kind="ExternalInput")
            cpar = dram.tile([128, 8], F32, kind="ExternalInput")
            outp = dram.tile([NIMG, 64, PIX], F16, kind="ExternalOutput")
            names.update(xta=xta.name, xsh=xsh.name, xin=xin.name,
                         w1a=w1a.name, w1bp=w1bp.name, w1bs=w1bs.name,
                         w2a8=w2a8.name, cpar=cpar.name, outp=outp.name)

            with tc.tile_pool(name="dramw", bufs=1, space="DRAM") as dramw, \
                 tc.tile_pool(name="wsb", bufs=1) as wsb, \
                 tc.tile_pool(name="ys", bufs=8) as yspool, \
                 tc.tile_pool(name="plane", bufs=3) as plpool, \
                 tc.tile_pool(name="planeh", bufs=2) as phpool, \
                 tc.tile_pool(name="hfp", bufs=2) as hf, \
                 tc.tile_pool(name="tiny", bufs=40) as tiny, \
                 tc.tile_pool(name="ps", bufs=7, space="PSUM") as ps:

                # ---- static parameter loads (w1a first: needed by matmul 0;
                # the rest issue on other queues / after first planes)
                w1as = wsb.tile([128, 9 * 64], F16, tag="w1a")
                nc.sync.dma_start(w1as[:], w1a[:])
                w1bps = wsb.tile([128, 3 * 64], F16, tag="w1bp")
                nc.scalar.dma_start(w1bps[:], w1bp[:])
                w1bss = wsb.tile([128, 3 * 64], F16, tag="w1bs")
                nc.scalar.dma_start(w1bss[:], w1bs[:])
                w2a8s = wsb.tile([128, 2 * 9 * 64], F8, tag="w2a8")
                nc.scalar.dma_start(w2a8s[:], w2a8[:])
                w2a8r = w2a8s.rearrange("p (two f) -> p two f", two=2)
                cpars = wsb.tile([128, 8], F32, tag="cpar")
                nc.scalar.dma_start(cpars[:], cpar[:])
                lhs_map = {"w1a": w1as, "w1bp": w1bps, "w1bs": w1bss}
                sums1 = wsb.tile([128, 56], F32, tag="sums1")
                sums1q = wsb.tile([128, 56], F32, tag="sums1q")
                # conv2 sums: img-A cols 0:56, img-B cols 56:112, all in
                # partitions 0:64 (DoubleRow psum lives at partitions 0:64);
                # partitions 64:128 zeroed so the stats shuffle-add is a no-op
                sums2 = wsb.tile([128, 112], F32, tag="sums2")
                sums2q = wsb.tile([128, 56], F32, tag="sums2q")
                nc.vector.memset(sums2[64:128, :], 0.0)
                if PHASES < 2:
                    nc.vector.memset(sums2[0:64, :], 0.0)
                    nc.vector.memset(sums2q[:], 0.0)

                def conv_img_pair(pcA, pcB, phA, phB, ops, dst_strip, sums_t,
                                  sumsq_t, pcol):
                    """One image pair -> 7 chunks in two waves (4+3); ops
                    outer within a wave so consecutive matmuls hit different
                    PSUM banks and weight loads amortize; wave evacuations
                    overlap the next wave's matmuls."""
                    plmap = {"pc": [pcA, pcB]}
                    if phA is not None:
                        plmap["ph"] = [phA, phB]
                    for k in plmap:
                        plmap[k] = [p.rearrange("p (r w) -> p r w", w=HP)
                                    for p in plmap[k]]
                    nops = len(ops)
                    for wave in (range(0, 4), range(4, 7)):
                        pts = {}
                        for cth in wave:
                            pts[cth] = ps.tile([128, CHW], F32, tag="ps",
                                               bufs=7, name=f"psum{cth}")
                        for oi, (pk, ln, co, di, dj) in enumerate(ops):
                            lt = lhs_map[ln]
                            for cth in wave:
                                r0 = 8 * cth + di
                                for j in range(2):
                                    plr = plmap[pk][j]
                                    rhs = plr[:, r0:r0 + 8, dj:dj + W]
                                    out = pts[cth][64 * j:64 * (j + 1), :] \
                                        .rearrange("p (r w) -> p r w", r=8)
                                    nc.tensor.matmul(
                                        out, lt[:, co:co + 64], rhs,
                                        start=(oi == 0), stop=(oi == nops - 1),
                                        tile_position=(0, 64 * j),
                                        skip_group_check=True)
                        for cth in wave:
                            nc.scalar.activation(
                                dst_strip[:, CHW * cth:CHW * (cth + 1)],
                                pts[cth][:], AF.Copy,
                                accum_out=sums_t[:, pcol * 7 + cth:pcol * 7 + cth + 1])
                            jk = ps.tile([128, CHW], F32, tag="psjk", bufs=1,
                                         name="psjk")
                            sl = dst_strip[:, CHW * cth:CHW * (cth + 1)]
                            nc.vector.scalar_tensor_tensor(
                                jk[:], sl, 1.0, sl, AO.bypass, AO.mult,
                                accum_out=sumsq_t[:, pcol * 7 + cth:pcol * 7 + cth + 1])

                def conv2_pair_fp8(pl8A, pl8B, dst_strip, pcol):
                    """conv2 via fp8 DoubleRow: per tap one matmul computes
                    all four E4M3 weight terms (pairs interleaved on the 2-dim,
                    term pair 3/4 pre-scaled 2^9 against spike copies *2^-9 on
                    partitions 64:128). DoubleRow psum must live at partitions
                    0:64, so each image gets its own psum tile; image B is
                    evacuated via a partition-moving DMA."""
                    plr8 = [p.rearrange("p (r w) -> p r w", w=HP)
                            for p in (pl8A, pl8B)]
                    for wave in (range(0, 3), range(3, 5), range(5, 7)):
                        pts = {}
                        for cth in wave:
                            for j in range(2):
                                pts[(cth, j)] = ps.tile(
                                    [128, CHW], F32, tag="ps", bufs=7,
                                    name=f"ps8_{cth}_{j}")
                        for a in range(9):
                            di, dj = a // 3, a % 3
                            lhs = w2a8r[:, :, a * 64:(a + 1) * 64]
                            for cth in wave:
                                r0 = 8 * cth + di
                                for j in range(2):
                                    rhs = plr8[j][:, r0:r0 + 8, dj:dj + W] \
                                        .unsqueeze(1) \
                                        .broadcast_to([128, 2, 8, W])
                                    out = pts[(cth, j)][0:64, :] \
                                        .rearrange("p (r w) -> p r w", r=8)
                                    nc.tensor.matmul(
                                        out, lhs, rhs,
                                        start=(a == 0), stop=(a == 8),
                                        perf_mode=PM.DoubleRow,
                                        tile_position=(0, 0),
                                        skip_group_check=True)
                        for cth in wave:
                            ccol = slice(CHW * cth, CHW * (cth + 1))
                            nc.scalar.activation(
                                dst_strip[0:64, ccol], pts[(cth, 0)][0:64, :],
                                AF.Copy,
                                accum_out=sums2[0:64, pcol * 7 + cth:pcol * 7 + cth + 1])
                            tmpb = hf.tile([128, CHW], F32, tag="tmpb",
                                           bufs=1)
                            nc.scalar.activation(
                                tmpb[0:64, :], pts[(cth, 1)][0:64, :],
                                AF.Copy,
                                accum_out=sums2[0:64, 56 + pcol * 7 + cth:56 + pcol * 7 + cth + 1])
                            nc.sync.dma_start(dst_strip[64:128, ccol],
                                              tmpb[0:64, :])
                            sl = dst_strip[:, ccol]
                            jk = ps.tile([128, CHW], F32, tag="psjk", bufs=1,
                                         name="psjk")
                            nc.vector.scalar_tensor_tensor(
                                jk[:], sl, 1.0, sl, AO.bypass, AO.mult,
                                accum_out=sums2q[:, pcol * 7 + cth:pcol * 7 + cth + 1])

                # ================= phase A: conv1 =================
                y1s = []
                for p in range(NPAIR):
                    tt_, bp = p // 2, p % 2
                    iA = tt_ * 4 + bp * 2
                    pcs, phs = [], []
                    for j in range(2):
                        i = iA + j
                        pc = plpool.tile([128, PP], F16, tag="ta", bufs=2)
                        nc.sync.dma_start(
                            pc[:], xta[i].rearrange("a b q -> (a b) q"))
                        pcs.append(pc)
                        ph = phpool.tile([128, PP], F16, tag="tb")
                        nc.gpsimd.dma_start(ph[0:64, :], xta[i, 0])
                        nc.gpsimd.dma_start(ph[64:128, :], xsh[i])
                        phs.append(ph)
                    strip = yspool.tile([128, PIX], F32, tag="ys")
                    y1s.append(strip)
                    conv_img_pair(pcs[0], pcs[1], phs[0], phs[1], ops1, strip,
                                  sums1, sums1q, p)

                # ---- stats1 allreduce
                cc1i = dramw.tile([128, 2], F32)
                cc1o = dramw.tile([128, 2], F32, addr_space="Shared")
                acc1 = tiny.tile([128, 2], F32, tag="acc")
                nc.vector.tensor_reduce(acc1[:, 0:1], sums1[:], AX.X, AO.add)
                nc.vector.tensor_reduce(acc1[:, 1:2], sums1q[:], AX.X, AO.add)
                nc.sync.dma_start(cc1i[:], acc1[:])
                if NO_CC:
                    nc.sync.dma_start(cc1o[:], cc1i[:])
                else:
                    nc.gpsimd.collective_compute(
                        "AllReduce", AO.add, ins=[cc1i[:]], outs=[cc1o[:]],
                        replica_groups=[list(range(NCORES))])
                g1 = tiny.tile([128, 2], F32, tag="acc")
                nc.sync.dma_start(g1[:], cc1o[:])

                def stats_block(g, gamma, beta, rga, rgam, alpha):
                    gr = tiny.tile([128, 2], F32, tag="acc")
                    nc.sync.dma_start(gr[0:64, :], g[64:128, :])
                    nc.sync.dma_start(gr[64:128, :], g[0:64, :])
                    tot = tiny.tile([128, 2], F32, tag="acc")
                    nc.vector.tensor_tensor(tot[:], g[:], gr[:], AO.add)
                    mean = tiny.tile([128, 1], F32, tag="t1")
                    nc.vector.tensor_scalar(mean[:], tot[:, 0:1], 1.0 / NG,
                                            None, AO.mult)
                    msq = tiny.tile([128, 1], F32, tag="t1")
                    nc.vector.tensor_scalar(msq[:], tot[:, 1:2], 1.0 / NG,
                                            None, AO.mult)
                    m2 = tiny.tile([128, 1], F32, tag="t1")
                    nc.vector.scalar_tensor_tensor(m2[:], mean[:], 1.0, mean[:],
                                                   AO.bypass, AO.mult)
                    var = tiny.tile([128, 1], F32, tag="t1")
                    nc.vector.tensor_tensor(var[:], msq[:], m2[:], AO.subtract)
                    epst = tiny.tile([128, 1], F32, tag="t1")
                    nc.vector.memset(epst[:], EPS)
                    std = tiny.tile([128, 1], F32, tag="t1")
                    nc.scalar.activation(std[:], var[:], AF.Sqrt, bias=epst[:])
                    rstd = tiny.tile([128, 1], F32, tag="t1")
                    nc.vector.reciprocal(rstd[:], std[:])
                    sc = tiny.tile([128, 1], F32, tag="t1")
                    nc.vector.tensor_tensor(sc[:], gamma, rstd[:], AO.mult)
                    nmsc = tiny.tile([128, 1], F32, tag="t1")
                    nc.vector.scalar_tensor_tensor(nmsc[:], mean[:], -1.0, sc[:],
                                                   AO.mult, AO.mult)
                    bi = tiny.tile([128, 1], F32, tag="t1")
                    nc.vector.tensor_tensor(bi[:], beta, nmsc[:], AO.add)
                    stdrg = tiny.tile([128, 1], F32, tag="t1")
                    nc.vector.tensor_tensor(stdrg[:], std[:], rga, AO.mult)
                    nbst = tiny.tile([128, 1], F32, tag="t1")
                    nc.vector.scalar_tensor_tensor(nbst[:], bi[:], -alpha,
                                                   stdrg[:], AO.mult, AO.mult)
                    th = tiny.tile([128, 1], F32, tag="t1")
                    nc.vector.tensor_tensor(th[:], stdrg[:], nbst[:], AO.add)
                    bstd = tiny.tile([128, 1], F32, tag="t1")
                    nc.vector.tensor_tensor(bstd[:], bi[:], std[:], AO.mult)
                    gamv = tiny.tile([128, 1], F32, tag="t1")
                    nc.vector.tensor_tensor(gamv[:], bstd[:], rgam, AO.mult)
                    rscv = tiny.tile([128, 1], F32, tag="t1")
                    nc.vector.tensor_tensor(rscv[:], std[:], rgam, AO.mult)
                    gmw = tiny.tile([128, 1], F32, tag="t1")
                    nc.vector.tensor_scalar(gmw[:], gamv[:], 1.0 - alpha, None,
                                            AO.mult)
                    return th, gamv, rscv, gmw

                th1, gm1, _rsc1, gmw1 = stats_block(
                    g1, cpars[:, 0:1], cpars[:, 1:2], cpars[:, 4:5],
                    cpars[:, 6:7], alpha1)

                # ============ phase B + C: LIF1 + conv2 ============
                # spikes + plane DMAs first (conv2 start latency), P-updates
                # after; wv on Act via gmw1; spikes in fp8 with a *2^-9 copy
                # on partitions 64:128 feeding DoubleRow term pair 3/4.
                y2s = [None] * NPAIR
                ta8_gen = [0]
                for bp in range(2 if PHASES >= 2 else 0):
                    Pprev = [None] * NQ
                    for t in range(1, 5):
                        p = (t - 1) * 2 + bp
                        tas_pair = []
                        for j in range(2):
                            tas = plpool.tile([128, PP], F8, tag="ta8", bufs=4)
                            if ta8_gen[0] < 4:
                                # fresh buffer: zero once; later generations
                                # keep zero borders (DMAs write interior only)
                                nc.gpsimd.memset(tas[:], 0.0)
                            ta8_gen[0] += 1
                            tas_pair.append(tas)
                        qas = []
                        for hq in range(NQ):
                            off = QL * hq
                            ysl = y1s[p][:, off:off + QL]
                            if t == 1:
                                qa = ysl
                            else:
                                q = hf.tile([128, QL], F32, tag="q2", bufs=4)
                                nc.gpsimd.tensor_tensor(q[:], ysl,
                                                        Pprev[hq][:], AO.add)
                                qa = q[:]
                            qas.append(qa)
                            s8 = hf.tile([128, QL], F8, tag="s8", bufs=3)
                            nc.vector.tensor_scalar(s8[:], qa, th1[:],
                                                    None, AO.is_ge)
                            s8s = hf.tile([128, QL], F8, tag="s8s", bufs=2)
                            nc.vector.tensor_scalar(s8s[:], qa, th1[:],
                                                    2.0 ** -9, AO.is_ge,
                                                    AO.mult)
                            for j in range(2):
                                tasr = tas_pair[j].rearrange(
                                    "p (r w) -> p r w", w=HP)
                                dsti = tasr[:, 1 + 14 * hq:1 + 14 * (hq + 1),
                                            1:1 + W]
                                srcs = s8[64 * j:64 * (j + 1), :] \
                                    .rearrange("p (r w) -> p r w", w=W)
                                srcss = s8s[64 * j:64 * (j + 1), :] \
                                    .rearrange("p (r w) -> p r w", w=W)
                                nc.sync.dma_start(dsti[0:64], srcs)
                                nc.sync.dma_start(dsti[64:128], srcss)
                        if t < 4:
                            for hq in range(NQ):
                                qa = qas[hq]
                                wv = hf.tile([128, QL], F32, tag="wv", bufs=2)
                                nc.scalar.activation(wv[:], qa, AF.Identity,
                                                     bias=gmw1[:],
                                                     scale=1.0 - alpha1)
                                Pn = hf.tile([128, QL], F32, tag="pp",
                                             bufs=8)
                                nc.vector.scalar_tensor_tensor(
                                    Pn[:], qa, th1[:], wv[:], AO.is_lt,
                                    AO.mult)
                                Pprev[hq] = Pn
                        strip2 = yspool.tile([128, PIX], F32, tag="ys")
                        y2s[p] = strip2
                        conv2_pair_fp8(tas_pair[0], tas_pair[1], strip2, p)

                # ---- stats2 allreduce
                cc2i = dramw.tile([128, 2], F32)
                cc2o = dramw.tile([128, 2], F32, addr_space="Shared")
                acc2 = tiny.tile([128, 2], F32, tag="acc")
                nc.vector.tensor_reduce(acc2[:, 0:1], sums2[:], AX.X, AO.add)
                nc.vector.tensor_reduce(acc2[:, 1:2], sums2q[:], AX.X, AO.add)
                nc.sync.dma_start(cc2i[:], acc2[:])
                if NO_CC:
                    nc.sync.dma_start(cc2o[:], cc2i[:])
                else:
                    nc.gpsimd.collective_compute(
                        "AllReduce", AO.add, ins=[cc2i[:]], outs=[cc2o[:]],
                        replica_groups=[list(range(NCORES))])
                g2 = tiny.tile([128, 2], F32, tag="acc")
                nc.sync.dma_start(g2[:], cc2o[:])
                th2, gm2, rsc2, gmw2 = stats_block(
                    g2, cpars[:, 2:3], cpars[:, 3:4], cpars[:, 5:6],
                    cpars[:, 7:8], alpha2)
                nth2 = tiny.tile([128, 1], F32, tag="t1")
                nc.vector.tensor_scalar(nth2[:], th2[:], -1.0, None, AO.mult)

                # ============ phase D: residual + LIF2 ============
                # spike = Sign(q2 - th2) in fp16; host maps sign>=0 -> 1.
                # t-major so the 8 (bp,hq-pair) chains interleave; z on
                # GpSimd, q2 fused stt on DVE, spike on Act, out via Act DGE.
                Pprev2 = {}
                pend_pn = []
                pend_out = []
                for t in range(1 if PHASES >= 3 else 5, 5):
                    for bp in range(2):
                        p = (t - 1) * 2 + bp
                        iA = (t - 1) * 4 + bp * 2
                        for hq in range(NQ):
                            off = QL * hq
                            xs = hf.tile([128, QL], F32, tag="xs", bufs=3)
                            nc.sync.dma_start(
                                xs[:], xin[iA:iA + 2, :, off:off + QL]
                                .rearrange("a b q -> (a b) q"))
                            if t == 1:
                                zv = y2s[p][:, off:off + QL]
                            else:
                                z = hf.tile([128, QL], F32, tag="q2", bufs=4)
                                nc.gpsimd.tensor_tensor(
                                    z[:], y2s[p][:, off:off + QL],
                                    Pprev2[(bp, hq)][:], AO.add)
                                zv = z[:]
                            q2 = hf.tile([128, QL], F32, tag="q2", bufs=4)
                            nc.vector.scalar_tensor_tensor(
                                q2[:], xs[:], rsc2[:], zv, AO.mult, AO.add)
                            q2v = q2[:]
                            ot = hf.tile([128, QL], F16, tag="s1t", bufs=2)
                            nc.scalar.activation(ot[:], q2v, AF.Sign,
                                                 bias=nth2[:])
                            # defer the out-DMA by one chain so its wait on ot
                            # never blocks the Pool SEQ ahead of z-adds
                            pend_out.append((iA, off, ot))
                            if len(pend_out) > 1:
                                iAp, offp, otp = pend_out.pop(0)
                                nc.scalar.dma_start(
                                    outp[iAp:iAp + 2, :, offp:offp + QL]
                                    .rearrange("a b q -> (a b) q"), otp[:])
                            if t < 4:
                                wv2 = hf.tile([128, QL], F32, tag="wv",
                                              bufs=2)
                                nc.scalar.activation(wv2[:], q2v, AF.Identity,
                                                     bias=gmw2[:],
                                                     scale=1.0 - alpha2)
                                # defer Pn by one chain so the DVE queue head
                                # never waits on this chain's wv2 (Act)
                                pend_pn.append((bp, hq, wv2, q2))
                                if len(pend_pn) > 1:
                                    bpp, hqp, wvp, q2p = pend_pn.pop(0)
                                    Pn = hf.tile([128, QL], F32, tag="pp",
                                                 bufs=8)
                                    nc.vector.scalar_tensor_tensor(
                                        Pn[:], q2p[:], th2[:], wvp[:],
                                        AO.is_lt, AO.mult)
                                    Pprev2[(bpp, hqp)] = Pn
                    while pend_pn:
                        bpp, hqp, wvp, q2p = pend_pn.pop(0)
                        Pn = hf.tile([128, QL], F32, tag="pp", bufs=8)
                        nc.vector.scalar_tensor_tensor(
                            Pn[:], q2p[:], th2[:], wvp[:], AO.is_lt, AO.mult)
                        Pprev2[(bpp, hqp)] = Pn
                while pend_out:
                    iAp, offp, otp = pend_out.pop(0)
                    nc.scalar.dma_start(
                        outp[iAp:iAp + 2, :, offp:offp + QL]
                        .rearrange("a b q -> (a b) q"), otp[:])

    nc.compile()
    return nc, names


def _sigmoid(x):
    return 1.0 / (1.0 + np.exp(-float(x)))


def prepare(x, conv1_w, bn1_gamma, bn1_beta, lif1_w, conv2_w, bn2_gamma,
            bn2_beta, lif2_w):
    x = np.ascontiguousarray(np.asarray(x, np.float32))
    conv1_w = np.asarray(conv1_w, np.float32)
    conv2_w = np.asarray(conv2_w, np.float32)

    a1 = _sigmoid(np.asarray(lif1_w).reshape(-1)[0])
    a2 = _sigmoid(np.asarray(lif2_w).reshape(-1)[0])

    key = (round(a1, 12), round(a2, 12))
    if key not in _prog_cache:
        _prog_cache[key] = _build(a1, a2)
    nc, names = _prog_cache[key]

    # fp16 hi/lo split of x, padded planes (encoding only; exact split)
    xh = x.astype(np.float16)
    xl = (x - xh.astype(np.float32)).astype(np.float16)
    xpad = np.zeros((T, B, C, 2, HP, HP), np.float16)
    xpad[:, :, :, 0, 1:57, 1:57] = xh
    xpad[:, :, :, 1, 1:57, 1:57] = xl
    xpad = np.ascontiguousarray(xpad.transpose(0, 1, 3, 2, 4, 5))  # t,b,2,c,hp,hp
    # xhi shifted one column left (tap dj=1 via partitions 64:128)
    xshp = np.zeros((T, B, C, HP, HP), np.float16)
    xshp[:, :, :, 1:57, 0:56] = xh

    import ml_dtypes
    F8NP = ml_dtypes.float8_e4m3

    w1h = conv1_w.astype(np.float16)
    w1l = (conv1_w - w1h.astype(np.float32)).astype(np.float16)

    def tapstack(wtop, wbot):
        out = np.zeros((128, 9 * 64), np.float16)
        for a in range(9):
            di, dj = a // 3, a % 3
            out[0:64, a * 64:(a + 1) * 64] = wtop[:, :, di, dj].T
            out[64:128, a * 64:(a + 1) * 64] = wbot[:, :, di, dj].T
        return out

    w1a_np = tapstack(w1h, w1h)
    w1bp_np = np.zeros((128, 3 * 64), np.float16)
    w1bs_np = np.zeros((128, 3 * 64), np.float16)
    for di in range(3):
        w1bp_np[0:64, di * 64:(di + 1) * 64] = w1l[:, :, di, 0].T
        w1bp_np[64:128, di * 64:(di + 1) * 64] = w1l[:, :, di, 1].T
        w1bs_np[0:64, di * 64:(di + 1) * 64] = w1l[:, :, di, 2].T

    # conv2 weights: 4-term greedy E4M3 decomposition; terms 3/4 stored
    # scaled by 2^9 (device spikes *2^-9 on partitions 64:128 compensate)
    w2d = conv2_w.astype(np.float64)
    t1 = w2d.astype(F8NP)
    r = w2d - t1.astype(np.float64)
    t2 = r.astype(F8NP)
    r = r - t2.astype(np.float64)
    t3 = (r * 512.0).astype(F8NP)
    r = r - t3.astype(np.float64) / 512.0
    t4 = (r * 512.0).astype(F8NP)
    w2a8_np = np.zeros((128, 2, 9 * 64), F8NP)
    for a in range(9):
        di, dj = a // 3, a % 3
        w2a8_np[0:64, 0, a * 64:(a + 1) * 64] = t1[:, :, di, dj].T
        w2a8_np[0:64, 1, a * 64:(a + 1) * 64] = t2[:, :, di, dj].T
        w2a8_np[64:128, 0, a * 64:(a + 1) * 64] = t3[:, :, di, dj].T
        w2a8_np[64:128, 1, a * 64:(a + 1) * 64] = t4[:, :, di, dj].T
    w2a8_np = np.ascontiguousarray(w2a8_np.reshape(128, 2 * 9 * 64))

    def dup(v):
        v = np.asarray(v, np.float32).reshape(64)
        return np.concatenate([v, v])

    cpar_np = np.zeros((128, 8), np.float32)
    cpar_np[:, 0] = dup(bn1_gamma)
    cpar_np[:, 1] = dup(bn1_beta)
    cpar_np[:, 2] = dup(bn2_gamma)
    cpar_np[:, 3] = dup(bn2_beta)
    cpar_np[:, 4] = 1.0 / (a1 * dup(bn1_gamma))
    cpar_np[:, 5] = 1.0 / (a2 * dup(bn2_gamma))
    cpar_np[:, 6] = 1.0 / dup(bn1_gamma)
    cpar_np[:, 7] = 1.0 / dup(bn2_gamma)

    in_maps = []
    for k in range(NCORES):
        xta_np = np.ascontiguousarray(
            xpad[:, 4 * k:4 * k + 4].reshape(NIMG, 2, 64, PP))
        xsh_np = np.ascontiguousarray(
            xshp[:, 4 * k:4 * k + 4].reshape(NIMG, 64, PP))
        xin_np = np.ascontiguousarray(
            x[:, 4 * k:4 * k + 4].reshape(NIMG, 64, PIX))
        in_maps.append({
            names['xta']: xta_np,
            names['xsh']: xsh_np,
            names['xin']: xin_np,
            names['w1a']: w1a_np,
            names['w1bp']: w1bp_np,
            names['w1bs']: w1bs_np,
            names['w2a8']: w2a8_np,
            names['cpar']: cpar_np,
        })

    return nc, names, in_maps


def kernel(**inputs):
    from concourse.bass_utils import run_bass_kernel_spmd
    nc, names, in_maps = prepare(**inputs)
    res = run_bass_kernel_spmd(nc, in_maps, core_ids=list(range(NCORES)))
    global LAST_RES, LAST_NAMES
    LAST_RES, LAST_NAMES = res, names
    out = np.empty((T, B, C, H, W), np.float32)
    for k in range(NCORES):
        o = res.results[k][names['outp']]
        s = (o.astype(np.float32) >= 0.0).astype(np.float32)
        out[:, 4 * k:4 * k + 4] = s.reshape(T, BL, C, H, W)
    return out


if __name__ == "__main__":
    rng = np.random.default_rng(0)
    xs = rng.standard_normal((T, B, C, H, W)).astype(np.float32)
    w1 = (rng.standard_normal((64, 64, 3, 3)) * 0.05).astype(np.float32)
    w2 = (rng.standard_normal((64, 64, 3, 3)) * 0.05).astype(np.float32)
    o = kernel(x=xs, conv1_w=w1, bn1_gamma=np.ones(64, np.float32),
               bn1_beta=np.zeros(64, np.float32),
               lif1_w=np.zeros(1, np.float32), conv2_w=w2,
               bn2_gamma=np.ones(64, np.float32),
               bn2_beta=np.zeros(64, np.float32),
               lif2_w=np.zeros(1, np.float32))
    print("ran:", o.shape, float(o.mean()))


# revision 38
# speedup vs baseline: 1.1436x; 1.1436x over previous
"""Trainium2 Bass kernel for nn_BasicBlock (spiking CNN block).

Sharding: data-parallel over batch B across 8 NeuronCores (4 batch x 4
timesteps = 16 images per core); BN batch stats via tiny AllReduce.

Per core:
- conv1: 3x3 taps as TensorEngine matmuls in fp16 hi/lo split arithmetic
  (~fp32 accuracy at fp16 speed). Hi pass: per tap [W1hi;W1hi] x [xhi;xlo]
  (K=128, 9 matmuls). Lo pass tap-paired: a second plane copy holds xhi
  shifted one column, so [W1lo(di,0);W1lo(di,1)] x [xhi;xhi_sh] computes two
  taps per matmul (3 pair + 3 single = 6 matmuls instead of 9).
- BN stats (sum / sum-of-squares) accumulated during PSUM evacuation
  (ScalarE copy w/ accum_out; DVE square pass), all-reduced across cores.
- PLIF scan in "q-space" (conv-output units): BN scale/bias folded into
  per-channel threshold theta / constants, so no per-element BN apply.
- conv2 consumes exact 0/1 spikes in fp16: per tap [W2hi;W2lo] x [s1;s1]
  (K=128) gives both split terms in one matmul.
- Residual + LIF2: fused r = x*rsc2 + y2 on GpSimd, spike emitted as
  Sign(q2 - th2) on the Act engine in fp16; host maps sign>=0 -> 0/1.
"""
import sys
sys.path.insert(0, '/opt/trn_rl_repo')

import numpy as np

T, B, C, H, W = 4, 32, 64, 56, 56
NCORES = 8
BL = B // NCORES            # 4 local batch samples
NIMG = T * BL               # 16 images per core
HP = W + 2                  # 58
PP = HP * HP                # 3364 padded pixels
PIX = H * W                 # 3136
NCH = 7                     # conv chunks per image (8 rows each)
CHW = 8 * W                 # 448
NPAIR = 8                   # image pairs per core
EPS = 1e-5
NG = float((T * B) * PIX)   # 401408
QL = 14 * W                 # LIF quarter-strip length (784)
NQ = 4

_prog_cache = {}
DBG = False
NO_CC = False
PHASES = 3
TRACE = False
LAST_RES = None
LAST_NAMES = None
LAST_EXEC_NS = None


def _build(alpha1, alpha2):
    import concourse.mybir as mybir
    import concourse.tile as tile
    from concourse import bacc

    F32 = mybir.dt.float32
    F16 = mybir.dt.float16
    F8 = mybir.dt.float8e4
    AO = mybir.AluOpType
    AF = mybir.ActivationFunctionType
    AX = mybir.AxisListType
    PM = mybir.MatmulPerfMode

    nc = bacc.Bacc(None, target_bir_lowering=False)
    names = {}

    # conv1 op list: (plane_key, lhs_name, lhs_col, di, dj)
    ops1 = [("pc", "w1a", a * 64, a // 3, a % 3) for a in range(9)] \
        + [("ph", "w1bp", di * 64, di, 0) for di in range(3)] \
        + [("ph", "w1bs", di * 64, di, 2) for di in range(3)]

    with tile.TileContext(nc) as tc:
        with tc.tile_pool(name="dram", bufs=1, space="DRAM") as dram:
            xta = dram.tile([NIMG, 2, 64, PP], F16, kind="ExternalInput")
            xsh = dram.tile([NIMG, 64, PP], F16, kind="ExternalInput")
            xin = dram.tile([NIMG, 64, PIX], F32, kind="ExternalInput")
            w1a = dram.tile([128, 9 * 64], F16, kind="ExternalInput")
            w1bp = dram.tile([128, 3 * 64], F16, kind="ExternalInput")
            w1bs = dram.tile([128, 3 * 64], F16, kind="ExternalInput")
            w2a8 = dram.tile([128, 2 * 9 * 64], F8, kind="ExternalInput")
            cpar = dram.tile([128, 8], F32, kind="ExternalInput")
            outp = dram.tile([NIMG, 64, PIX], F16, kind="ExternalOutput")
            names.update(xta=xta.name, xsh=xsh.name, xin=xin.name,
                         w1a=w1a.name, w1bp=w1bp.name, w1bs=w1bs.name,
                         w2a8=w2a8.name, cpar=cpar.name, outp=outp.name)

            with tc.tile_pool(name="dramw", bufs=1, space="DRAM") as dramw, \
                 tc.tile_pool(name="wsb", bufs=1) as wsb, \
                 tc.tile_pool(name="ys", bufs=8) as yspool, \
                 tc.tile_pool(name="plane", bufs=3) as plpool, \
                 tc.tile_pool(name="planeh", bufs=2) as phpool, \
                 tc.tile_pool(name="hfp", bufs=2) as hf, \
                 tc.tile_pool(name="tiny", bufs=30) as tiny, \
                 tc.tile_pool(name="ps", bufs=7, space="PSUM") as ps:

                # ---- static parameter loads (w1a first: needed by matmul 0;
                # the rest issue on other queues / after first planes)
                w1as = wsb.tile([128, 9 * 64], F16, tag="w1a")
                nc.sync.dma_start(w1as[:], w1a[:])
                w1bps = wsb.tile([128, 3 * 64], F16, tag="w1bp")
                nc.scalar.dma_start(w1bps[:], w1bp[:])
                w1bss = wsb.tile([128, 3 * 64], F16, tag="w1bs")
                nc.scalar.dma_start(w1bss[:], w1bs[:])
                w2a8s = wsb.tile([128, 2 * 9 * 64], F8, tag="w2a8")
                nc.scalar.dma_start(w2a8s[:], w2a8[:])
                w2a8r = w2a8s.rearrange("p (two f) -> p two f", two=2)
                cpars = wsb.tile([128, 8], F32, tag="cpar")
                nc.scalar.dma_start(cpars[:], cpar[:])
                lhs_map = {"w1a": w1as, "w1bp": w1bps, "w1bs": w1bss}
                sums1 = wsb.tile([128, 56], F32, tag="sums1")
                sums1q = wsb.tile([128, 56], F32, tag="sums1q")
                # conv2 sums: img-A cols 0:56, img-B cols 56:112, all in
                # partitions 0:64 (DoubleRow psum lives at partitions 0:64);
                # partitions 64:128 zeroed so the stats shuffle-add is a no-op
                sums2 = wsb.tile([128, 112], F32, tag="sums2")
                sums2q = wsb.tile([128, 56], F32, tag="sums2q")
                nc.vector.memset(sums2[64:128, :], 0.0)
                if PHASES < 2:
                    nc.vector.memset(sums2[0:64, :], 0.0)
                    nc.vector.memset(sums2q[:], 0.0)

                def conv_img_pair(pcA, pcB, phA, phB, ops, dst_strip, sums_t,
                                  sumsq_t, pcol):
                    """One image pair -> 7 chunks in two waves (4+3); ops
                    outer within a wave so consecutive matmuls hit different
                    PSUM banks and weight loads amortize; wave evacuations
                    overlap the next wave's matmuls."""
                    plmap = {"pc": [pcA, pcB]}
                    if phA is not None:
                        plmap["ph"] = [phA, phB]
                    for k in plmap:
                        plmap[k] = [p.rearrange("p (r w) -> p r w", w=HP)
                                    for p in plmap[k]]
                    nops = len(ops)
                    for wave in (range(0, 4), range(4, 7)):
                        pts = {}
                        for cth in wave:
                            pts[cth] = ps.tile([128, CHW], F32, tag="ps",
                                               bufs=7, name=f"psum{cth}")
                        for oi, (pk, ln, co, di, dj) in enumerate(ops):
                            lt = lhs_map[ln]
                            for cth in wave:
                                r0 = 8 * cth + di
                                for j in range(2):
                                    plr = plmap[pk][j]
                                    rhs = plr[:, r0:r0 + 8, dj:dj + W]
                                    out = pts[cth][64 * j:64 * (j + 1), :] \
                                        .rearrange("p (r w) -> p r w", r=8)
                                    nc.tensor.matmul(
                                        out, lt[:, co:co + 64], rhs,
                                        start=(oi == 0), stop=(oi == nops - 1),
                                        tile_position=(0, 64 * j),
                                        skip_group_check=True)
                        for cth in wave:
                            nc.scalar.activation(
                                dst_strip[:, CHW * cth:CHW * (cth + 1)],
                                pts[cth][:], AF.Copy,
                                accum_out=sums_t[:, pcol * 7 + cth:pcol * 7 + cth + 1])
                            jk = ps.tile([128, CHW], F32, tag="psjk", bufs=1,
                                         name="psjk")
                            sl = dst_strip[:, CHW * cth:CHW * (cth + 1)]
                            nc.vector.scalar_tensor_tensor(
                                jk[:], sl, 1.0, sl, AO.bypass, AO.mult,
                                accum_out=sumsq_t[:, pcol * 7 + cth:pcol * 7 + cth + 1])

                def conv2_pair_fp8(pl8A, pl8B, dst_strip, pcol):
                    """conv2 via fp8 DoubleRow: per tap one matmul computes
                    all four E4M3 weight terms (pairs interleaved on the 2-dim,
                    term pair 3/4 pre-scaled 2^9 against spike copies *2^-9 on
                    partitions 64:128). DoubleRow psum must live at partitions
                    0:64, so each image gets its own psum tile; image B is
                    evacuated via a partition-moving DMA."""
                    plr8 = [p.rearrange("p (r w) -> p r w", w=HP)
                            for p in (pl8A, pl8B)]
                    for wave in (range(0, 3), range(3, 5), range(5, 7)):
                        pts = {}
                        for cth in wave:
                            for j in range(2):
                                pts[(cth, j)] = ps.tile(
                                    [128, CHW], F32, tag="ps", bufs=7,
                                    name=f"ps8_{cth}_{j}")
                        for a in range(9):
                            di, dj = a // 3, a % 3
                            lhs = w2a8r[:, :, a * 64:(a + 1) * 64]
                            for cth in wave:
                                r0 = 8 * cth + di
                                for j in range(2):
                                    rhs = plr8[j][:, r0:r0 + 8, dj:dj + W] \
                                        .unsqueeze(1) \
                                        .broadcast_to([128, 2, 8, W])
                                    out = pts[(cth, j)][0:64, :] \
                                        .rearrange("p (r w) -> p r w", r=8)
                                    nc.tensor.matmul(
                                        out, lhs, rhs,
                                        start=(a == 0), stop=(a == 8),
                                        perf_mode=PM.DoubleRow,
                                        tile_position=(0, 0),
                                        skip_group_check=True)
                        for cth in wave:
                            ccol = slice(CHW * cth, CHW * (cth + 1))
                            nc.scalar.activation(
                                dst_strip[0:64, ccol], pts[(cth, 0)][0:64, :],
                                AF.Copy,
                                accum_out=sums2[0:64, pcol * 7 + cth:pcol * 7 + cth + 1])
                            tmpb = hf.tile([128, CHW], F32, tag="tmpb",
                                           bufs=2)
                            nc.scalar.activation(
                                tmpb[0:64, :], pts[(cth, 1)][0:64, :],
                                AF.Copy,
                                accum_out=sums2[0:64, 56 + pcol * 7 + cth:56 + pcol * 7 + cth + 1])
                            nc.sync.dma_start(dst_strip[64:128, ccol],
                                              tmpb[0:64, :])
                            sl = dst_strip[:, ccol]
                            jk = ps.tile([128, CHW], F32, tag="psjk", bufs=1,
                                         name="psjk")
                            nc.vector.scalar_tensor_tensor(
                                jk[:], sl, 1.0, sl, AO.bypass, AO.mult,
                                accum_out=sums2q[:, pcol * 7 + cth:pcol * 7 + cth + 1])

                # ================= phase A: conv1 =================
                y1s = []
                for p in range(NPAIR):
                    tt_, bp = p // 2, p % 2
                    iA = tt_ * 4 + bp * 2
                    pcs, phs = [], []
                    for j in range(2):
                        i = iA + j
                        pc = plpool.tile([128, PP], F16, tag="ta", bufs=2)
                        nc.sync.dma_start(
                            pc[:], xta[i].rearrange("a b q -> (a b) q"))
                        pcs.append(pc)
                        ph = phpool.tile([128, PP], F16, tag="tb")
                        nc.gpsimd.dma_start(ph[0:64, :], xta[i, 0])
                        nc.gpsimd.dma_start(ph[64:128, :], xsh[i])
                        phs.append(ph)
                    strip = yspool.tile([128, PIX], F32, tag="ys")
                    y1s.append(strip)
                    conv_img_pair(pcs[0], pcs[1], phs[0], phs[1], ops1, strip,
                                  sums1, sums1q, p)

                # ---- stats1 allreduce
                cc1i = dramw.tile([128, 2], F32)
                cc1o = dramw.tile([128, 2], F32, addr_space="Shared")
                acc1 = tiny.tile([128, 2], F32, tag="acc")
                nc.vector.tensor_reduce(acc1[:, 0:1], sums1[:], AX.X, AO.add)
                nc.vector.tensor_reduce(acc1[:, 1:2], sums1q[:], AX.X, AO.add)
                nc.sync.dma_start(cc1i[:], acc1[:])
                if NO_CC:
                    nc.sync.dma_start(cc1o[:], cc1i[:])
                else:
                    nc.gpsimd.collective_compute(
                        "AllReduce", AO.add, ins=[cc1i[:]], outs=[cc1o[:]],
                        replica_groups=[list(range(NCORES))])
                g1 = tiny.tile([128, 2], F32, tag="acc")
                nc.sync.dma_start(g1[:], cc1o[:])

                def stats_block(g, gamma, beta, rga, rgam, alpha):
                    gr = tiny.tile([128, 2], F32, tag="acc")
                    nc.sync.dma_start(gr[0:64, :], g[64:128, :])
                    nc.sync.dma_start(gr[64:128, :], g[0:64, :])
                    tot = tiny.tile([128, 2], F32, tag="acc")
                    nc.vector.tensor_tensor(tot[:], g[:], gr[:], AO.add)
                    mean = tiny.tile([128, 1], F32, tag="t1")
                    nc.vector.tensor_scalar(mean[:], tot[:, 0:1], 1.0 / NG,
                                            None, AO.mult)
                    msq = tiny.tile([128, 1], F32, tag="t1")
                    nc.vector.tensor_scalar(msq[:], tot[:, 1:2], 1.0 / NG,
                                            None, AO.mult)
                    m2 = tiny.tile([128, 1], F32, tag="t1")
                    nc.vector.scalar_tensor_tensor(m2[:], mean[:], 1.0, mean[:],
                                                   AO.bypass, AO.mult)
                    var = tiny.tile([128, 1], F32, tag="t1")
                    nc.vector.tensor_tensor(var[:], msq[:], m2[:], AO.subtract)
                    epst = tiny.tile([128, 1], F32, tag="t1")
                    nc.vector.memset(epst[:], EPS)
                    std = tiny.tile([128, 1], F32, tag="t1")
                    nc.scalar.activation(std[:], var[:], AF.Sqrt, bias=epst[:])
                    rstd = tiny.tile([128, 1], F32, tag="t1")
                    nc.vector.reciprocal(rstd[:], std[:])
                    sc = tiny.tile([128, 1], F32, tag="t1")
                    nc.vector.tensor_tensor(sc[:], gamma, rstd[:], AO.mult)
                    nmsc = tiny.tile([128, 1], F32, tag="t1")
                    nc.vector.scalar_tensor_tensor(nmsc[:], mean[:], -1.0, sc[:],
                                                   AO.mult, AO.mult)
                    bi = tiny.tile([128, 1], F32, tag="t1")
                    nc.vector.tensor_tensor(bi[:], beta, nmsc[:], AO.add)
                    stdrg = tiny.tile([128, 1], F32, tag="t1")
                    nc.vector.tensor_tensor(stdrg[:], std[:], rga, AO.mult)
                    nbst = tiny.tile([128, 1], F32, tag="t1")
                    nc.vector.scalar_tensor_tensor(nbst[:], bi[:], -alpha,
                                                   stdrg[:], AO.mult, AO.mult)
                    th = tiny.tile([128, 1], F32, tag="t1")
                    nc.vector.tensor_tensor(th[:], stdrg[:], nbst[:], AO.add)
                    bstd = tiny.tile([128, 1], F32, tag="t1")
                    nc.vector.tensor_tensor(bstd[:], bi[:], std[:], AO.mult)
                    gamv = tiny.tile([128, 1], F32, tag="t1")
                    nc.vector.tensor_tensor(gamv[:], bstd[:], rgam, AO.mult)
                    rscv = tiny.tile([128, 1], F32, tag="t1")
                    nc.vector.tensor_tensor(rscv[:], std[:], rgam, AO.mult)
                    gmw = tiny.tile([128, 1], F32, tag="t1")
                    nc.vector.tensor_scalar(gmw[:], gamv[:], 1.0 - alpha, None,
                                            AO.mult)
                    return th, gamv, rscv, gmw

                th1, gm1, _rsc1, gmw1 = stats_block(
                    g1, cpars[:, 0:1], cpars[:, 1:2], cpars[:, 4:5],
                    cpars[:, 6:7], alpha1)

                # ============ phase B + C: LIF1 + conv2 ============
                # spikes + plane DMAs first (conv2 start latency), P-updates
                # after; wv on Act via gmw1; spikes in fp8 with a *2^-9 copy
                # on partitions 64:128 feeding DoubleRow term pair 3/4.
                y2s = [None] * NPAIR
                ta8_gen = [0]
                for bp in range(2 if PHASES >= 2 else 0):
                    Pprev = [None] * NQ
                    for t in range(1, 5):
                        p = (t - 1) * 2 + bp
                        tas_pair = []
                        for j in range(2):
                            tas = plpool.tile([128, PP], F8, tag="ta8", bufs=4)
                            if ta8_gen[0] < 4:
                                # fresh buffer: zero once; later generations
                                # keep zero borders (DMAs write interior only)
                                nc.gpsimd.memset(tas[:], 0.0)
                            ta8_gen[0] += 1
                            tas_pair.append(tas)
                        qas = []
                        for hq in range(NQ):
                            off = QL * hq
                            ysl = y1s[p][:, off:off + QL]
                            if t == 1:
                                qa = ysl
                            else:
                                q = hf.tile([128, QL], F32, tag="q2", bufs=4)
                                nc.gpsimd.tensor_tensor(q[:], ysl,
                                                        Pprev[hq][:], AO.add)
                                qa = q[:]
                            qas.append(qa)
                            s8 = hf.tile([128, QL], F8, tag="s8", bufs=2)
                            nc.vector.tensor_scalar(s8[:], qa, th1[:],
                                                    None, AO.is_ge)
                            s8s = hf.tile([128, QL], F8, tag="s8s", bufs=2)
                            nc.vector.tensor_scalar(s8s[:], qa, th1[:],
                                                    2.0 ** -9, AO.is_ge,
                                                    AO.mult)
                            for j in range(2):
                                tasr = tas_pair[j].rearrange(
                                    "p (r w) -> p r w", w=HP)
                                dsti = tasr[:, 1 + 14 * hq:1 + 14 * (hq + 1),
                                            1:1 + W]
                                srcs = s8[64 * j:64 * (j + 1), :] \
                                    .rearrange("p (r w) -> p r w", w=W)
                                srcss = s8s[64 * j:64 * (j + 1), :] \
                                    .rearrange("p (r w) -> p r w", w=W)
                                nc.sync.dma_start(dsti[0:64], srcs)
                                nc.sync.dma_start(dsti[64:128], srcss)
                        if t < 4:
                            for hq in range(NQ):
                                qa = qas[hq]
                                wv = hf.tile([128, QL], F32, tag="wv", bufs=2)
                                nc.scalar.activation(wv[:], qa, AF.Identity,
                                                     bias=gmw1[:],
                                                     scale=1.0 - alpha1)
                                Pn = hf.tile([128, QL], F32, tag="pp",
                                             bufs=8)
                                nc.vector.scalar_tensor_tensor(
                                    Pn[:], qa, th1[:], wv[:], AO.is_lt,
                                    AO.mult)
                                Pprev[hq] = Pn
                        strip2 = yspool.tile([128, PIX], F32, tag="ys")
                        y2s[p] = strip2
                        conv2_pair_fp8(tas_pair[0], tas_pair[1], strip2, p)

                # ---- stats2 allreduce
                cc2i = dramw.tile([128, 2], F32)
                cc2o = dramw.tile([128, 2], F32, addr_space="Shared")
                acc2 = tiny.tile([128, 2], F32, tag="acc")
                nc.vector.tensor_reduce(acc2[:, 0:1], sums2[:], AX.X, AO.add)
                nc.vector.tensor_reduce(acc2[:, 1:2], sums2q[:], AX.X, AO.add)
                nc.sync.dma_start(cc2i[:], acc2[:])
                if NO_CC:
                    nc.sync.dma_start(cc2o[:], cc2i[:])
                else:
                    nc.gpsimd.collective_compute(
                        "AllReduce", AO.add, ins=[cc2i[:]], outs=[cc2o[:]],
                        replica_groups=[list(range(NCORES))])
                g2 = tiny.tile([128, 2], F32, tag="acc")
                nc.sync.dma_start(g2[:], cc2o[:])
                th2, gm2, rsc2, gmw2 = stats_block(
                    g2, cpars[:, 2:3], cpars[:, 3:4], cpars[:, 5:6],
                    cpars[:, 7:8], alpha2)
                nth2 = tiny.tile([128, 1], F32, tag="t1")
                nc.vector.tensor_scalar(nth2[:], th2[:], -1.0, None, AO.mult)

                # ============ phase D: residual + LIF2 ============
                # spike = Sign(q2 - th2) in fp16; host maps sign>=0 -> 1.
                # t-major so the 8 (bp,hq-pair) chains interleave; z on
                # GpSimd, q2 fused stt on DVE, spike on Act, out via Act DGE.
                Pprev2 = {}
                pend_pn = []
                pend_out = []
                for t in range(1 if PHASES >= 3 else 5, 5):
                    for bp in range(2):
                        p = (t - 1) * 2 + bp
                        iA = (t - 1) * 4 + bp * 2
                        for hq in range(NQ):
                            off = QL * hq
                            xs = hf.tile([128, QL], F32, tag="xs", bufs=3)
                            nc.sync.dma_start(
                                xs[:], xin[iA:iA + 2, :, off:off + QL]
                                .rearrange("a b q -> (a b) q"))
                            if t == 1:
                                zv = y2s[p][:, off:off + QL]
                            else:
                                z = hf.tile([128, QL], F32, tag="q2", bufs=4)
                                nc.gpsimd.tensor_tensor(
                                    z[:], y2s[p][:, off:off + QL],
                                    Pprev2[(bp, hq)][:], AO.add)
                                zv = z[:]
                            q2 = hf.tile([128, QL], F32, tag="q2", bufs=4)
                            nc.vector.scalar_tensor_tensor(
                                q2[:], xs[:], rsc2[:], zv, AO.mult, AO.add)
                            q2v = q2[:]
                            ot = hf.tile([128, QL], F16, tag="s1t", bufs=2)
                            nc.scalar.activation(ot[:], q2v, AF.Sign,
                                                 bias=nth2[:])
                            # defer the out-DMA by one chain so its wait on ot
                            # never blocks the Pool SEQ ahead of z-adds
                            pend_out.append((iA, off, ot))
                            if len(pend_out) > 1:
                                iAp, offp, otp = pend_out.pop(0)
                                nc.scalar.dma_start(
                                    outp[iAp:iAp + 2, :, offp:offp + QL]
                                    .rearrange("a b q -> (a b) q"), otp[:])
                            if t < 4:
                                wv2 = hf.tile([128, QL], F32, tag="wv",
                                              bufs=2)
                                nc.scalar.activation(wv2[:], q2v, AF.Identity,
                                                     bias=gmw2[:],
                                                     scale=1.0 - alpha2)
                                # defer Pn by one chain so the DVE queue head
                                # never waits on this chain's wv2 (Act)
                                pend_pn.append((bp, hq, wv2, q2))
                                if len(pend_pn) > 1:
                                    bpp, hqp, wvp, q2p = pend_pn.pop(0)
                                    Pn = hf.tile([128, QL], F32, tag="pp",
                                                 bufs=8)
                                    nc.vector.scalar_tensor_tensor(
                                        Pn[:], q2p[:], th2[:], wvp[:],
                                        AO.is_lt, AO.mult)
                                    Pprev2[(bpp, hqp)] = Pn
                    while pend_pn:
                        bpp, hqp, wvp, q2p = pend_pn.pop(0)
                        Pn = hf.tile([128, QL], F32, tag="pp", bufs=8)
                        nc.vector.scalar_tensor_tensor(
                            Pn[:], q2p[:], th2[:], wvp[:], AO.is_lt, AO.mult)
                        Pprev2[(bpp, hqp)] = Pn
                while pend_out:
                    iAp, offp, otp = pend_out.pop(0)
                    nc.scalar.dma_start(
                        outp[iAp:iAp + 2, :, offp:offp + QL]
                        .rearrange("a b q -> (a b) q"), otp[:])

    nc.compile()
    return nc, names


def _sigmoid(x):
    return 1.0 / (1.0 + np.exp(-float(x)))


def prepare(x, conv1_w, bn1_gamma, bn1_beta, lif1_w, conv2_w, bn2_gamma,
            bn2_beta, lif2_w):
    x = np.ascontiguousarray(np.asarray(x, np.float32))
    conv1_w = np.asarray(conv1_w, np.float32)
    conv2_w = np.asarray(conv2_w, np.float32)

    a1 = _sigmoid(np.asarray(lif1_w).reshape(-1)[0])
    a2 = _sigmoid(np.asarray(lif2_w).reshape(-1)[0])

    key = (round(a1, 12), round(a2, 12))
    if key not in _prog_cache:
        _prog_cache[key] = _build(a1, a2)
    nc, names = _prog_cache[key]

    # fp16 hi/lo split of x, padded planes (encoding only; exact split)
    xh = x.astype(np.float16)
    xl = (x - xh.astype(np.float32)).astype(np.float16)
    xpad = np.zeros((T, B, C, 2, HP, HP), np.float16)
    xpad[:, :, :, 0, 1:57, 1:57] = xh
    xpad[:, :, :, 1, 1:57, 1:57] = xl
    xpad = np.ascontiguousarray(xpad.transpose(0, 1, 3, 2, 4, 5))  # t,b,2,c,hp,hp
    # xhi shifted one column left (tap dj=1 via partitions 64:128)
    xshp = np.zeros((T, B, C, HP, HP), np.float16)
    xshp[:, :, :, 1:57, 0:56] = xh

    import ml_dtypes
    F8NP = ml_dtypes.float8_e4m3

    w1h = conv1_w.astype(np.float16)
    w1l = (conv1_w - w1h.astype(np.float32)).astype(np.float16)

    def tapstack(wtop, wbot):
        out = np.zeros((128, 9 * 64), np.float16)
        for a in range(9):
            di, dj = a // 3, a % 3
            out[0:64, a * 64:(a + 1) * 64] = wtop[:, :, di, dj].T
            out[64:128, a * 64:(a + 1) * 64] = wbot[:, :, di, dj].T
        return out

    w1a_np = tapstack(w1h, w1h)
    w1bp_np = np.zeros((128, 3 * 64), np.float16)
    w1bs_np = np.zeros((128, 3 * 64), np.float16)
    for di in range(3):
        w1bp_np[0:64, di * 64:(di + 1) * 64] = w1l[:, :, di, 0].T
        w1bp_np[64:128, di * 64:(di + 1) * 64] = w1l[:, :, di, 1].T
        w1bs_np[0:64, di * 64:(di + 1) * 64] = w1l[:, :, di, 2].T

    # conv2 weights: 4-term greedy E4M3 decomposition; terms 3/4 stored
    # scaled by 2^9 (device spikes *2^-9 on partitions 64:128 compensate)
    w2d = conv2_w.astype(np.float64)
    t1 = w2d.astype(F8NP)
    r = w2d - t1.astype(np.float64)
    t2 = r.astype(F8NP)
    r = r - t2.astype(np.float64)
    t3 = (r * 512.0).astype(F8NP)
    r = r - t3.astype(np.float64) / 512.0
    t4 = (r * 512.0).astype(F8NP)
    w2a8_np = np.zeros((128, 2, 9 * 64), F8NP)
    for a in range(9):
        di, dj = a // 3, a % 3
        w2a8_np[0:64, 0, a * 64:(a + 1) * 64] = t1[:, :, di, dj].T
        w2a8_np[0:64, 1, a * 64:(a + 1) * 64] = t2[:, :, di, dj].T
        w2a8_np[64:128, 0, a * 64:(a + 1) * 64] = t3[:, :, di, dj].T
        w2a8_np[64:128, 1, a * 64:(a + 1) * 64] = t4[:, :, di, dj].T
    w2a8_np = np.ascontiguousarray(w2a8_np.reshape(128, 2 * 9 * 64))

    def dup(v):
        v = np.asarray(v, np.float32).reshape(64)
        return np.concatenate([v, v])

    cpar_np = np.zeros((128, 8), np.float32)
    cpar_np[:, 0] = dup(bn1_gamma)
    cpar_np[:, 1] = dup(bn1_beta)
    cpar_np[:, 2] = dup(bn2_gamma)
    cpar_np[:, 3] = dup(bn2_beta)
    cpar_np[:, 4] = 1.0 / (a1 * dup(bn1_gamma))
    cpar_np[:, 5] = 1.0 / (a2 * dup(bn2_gamma))
    cpar_np[:, 6] = 1.0 / dup(bn1_gamma)
    cpar_np[:, 7] = 1.0 / dup(bn2_gamma)

    in_maps = []
    for k in range(NCORES):
        xta_np = np.ascontiguousarray(
            xpad[:, 4 * k:4 * k + 4].reshape(NIMG, 2, 64, PP))
        xsh_np = np.ascontiguousarray(
            xshp[:, 4 * k:4 * k + 4].reshape(NIMG, 64, PP))
        xin_np = np.ascontiguousarray(
            x[:, 4 * k:4 * k + 4].reshape(NIMG, 64, PIX))
        in_maps.append({
            names['xta']: xta_np,
            names['xsh']: xsh_np,
            names['xin']: xin_np,
            names['w1a']: w1a_np,
            names['w1bp']: w1bp_np,
            names['w1bs']: w1bs_np,
            names['w2a8']: w2a8_np,
            names['cpar']: cpar_np,
        })

    return nc, names, in_maps


def kernel(**inputs):
    from concourse.bass_utils import run_bass_kernel_spmd
    nc, names, in_maps = prepare(**inputs)
    res = run_bass_kernel_spmd(nc, in_maps, core_ids=list(range(NCORES)))
    global LAST_RES, LAST_NAMES
    LAST_RES, LAST_NAMES = res, names
    out = np.empty((T, B, C, H, W), np.float32)
    for k in range(NCORES):
        o = res.results[k][names['outp']]
        s = (o.astype(np.float32) >= 0.0).astype(np.float32)
        out[:, 4 * k:4 * k + 4] = s.reshape(T, BL, C, H, W)
    return out


if __name__ == "__main__":
    rng = np.random.default_rng(0)
    xs = rng.standard_normal((T, B, C, H, W)).astype(np.float32)
    w1 = (rng.standard_normal((64, 64, 3, 3)) * 0.05).astype(np.float32)
    w2 = (rng.standard_normal((64, 64, 3, 3)) * 0.05).astype(np.float32)
    o = kernel(x=xs, conv1_w=w1, bn1_gamma=np.ones(64, np.float32),
               bn1_beta=np.zeros(64, np.float32),
               lif1_w=np.zeros(1, np.float32), conv2_w=w2,
               bn2_gamma=np.ones(64, np.float32),
               bn2_beta=np.zeros(64, np.float32),
               lif2_w=np.zeros(1, np.float32))
    print("ran:", o.shape, float(o.mean()))


# revision 41
# speedup vs baseline: 1.1574x; 1.0120x over previous
"""Trainium2 Bass kernel for nn_BasicBlock (spiking CNN block).

Sharding: data-parallel over batch B across 8 NeuronCores (4 batch x 4
timesteps = 16 images per core); BN batch stats via tiny AllReduce.

Per core:
- conv1: 3x3 taps as TensorEngine matmuls in fp16 hi/lo split arithmetic
  (~fp32 accuracy at fp16 speed). Hi pass: per tap [W1hi;W1hi] x [xhi;xlo]
  (K=128, 9 matmuls). Lo pass tap-paired: a second plane copy holds xhi
  shifted one column, so [W1lo(di,0);W1lo(di,1)] x [xhi;xhi_sh] computes two
  taps per matmul (3 pair + 3 single = 6 matmuls instead of 9).
- BN stats (sum / sum-of-squares) accumulated during PSUM evacuation
  (ScalarE copy w/ accum_out; DVE square pass), all-reduced across cores.
- PLIF scan in "q-space" (conv-output units): BN scale/bias folded into
  per-channel threshold theta / constants, so no per-element BN apply.
- conv2 consumes exact 0/1 spikes in fp16: per tap [W2hi;W2lo] x [s1;s1]
  (K=128) gives both split terms in one matmul.
- Residual + LIF2: fused r = x*rsc2 + y2 on GpSimd, spike emitted as
  Sign(q2 - th2) on the Act engine in fp16; host maps sign>=0 -> 0/1.
"""
import sys
sys.path.insert(0, '/opt/trn_rl_repo')

import numpy as np

T, B, C, H, W = 4, 32, 64, 56, 56
NCORES = 8
BL = B // NCORES            # 4 local batch samples
NIMG = T * BL               # 16 images per core
HP = W + 2                  # 58
PP = HP * HP                # 3364 padded pixels
PIX = H * W                 # 3136
NCH = 7                     # conv chunks per image (8 rows each)
CHW = 8 * W                 # 448
NPAIR = 8                   # image pairs per core
EPS = 1e-5
NG = float((T * B) * PIX)   # 401408
QL = 14 * W                 # LIF quarter-strip length (784)
NQ = 4

_prog_cache = {}
DBG = False
NO_CC = False
PHASES = 3
TRACE = False
LAST_RES = None
LAST_NAMES = None
LAST_EXEC_NS = None


def _build(alpha1, alpha2):
    import concourse.mybir as mybir
    import concourse.tile as tile
    from concourse import bacc

    F32 = mybir.dt.float32
    F16 = mybir.dt.float16
    F8 = mybir.dt.float8e4
    AO = mybir.AluOpType
    AF = mybir.ActivationFunctionType
    AX = mybir.AxisListType
    PM = mybir.MatmulPerfMode

    nc = bacc.Bacc(None, target_bir_lowering=False)
    names = {}

    # conv1 op list: (plane_key, lhs_name, lhs_col, di, dj)
    ops1 = [("pc", "w1a", a * 64, a // 3, a % 3) for a in range(9)] \
        + [("ph", "w1bp", di * 64, di, 0) for di in range(3)] \
        + [("ph", "w1bs", di * 64, di, 2) for di in range(3)]

    with tile.TileContext(nc) as tc:
        with tc.tile_pool(name="dram", bufs=1, space="DRAM") as dram:
            xta = dram.tile([NIMG, 2, 64, PP], F16, kind="ExternalInput")
            xsh = dram.tile([NIMG, 64, PP], F16, kind="ExternalInput")
            xin = dram.tile([NIMG, 64, PIX], F32, kind="ExternalInput")
            w1a = dram.tile([128, 9 * 64], F16, kind="ExternalInput")
            w1bp = dram.tile([128, 3 * 64], F16, kind="ExternalInput")
            w1bs = dram.tile([128, 3 * 64], F16, kind="ExternalInput")
            w2a8 = dram.tile([128, 2 * 9 * 64], F8, kind="ExternalInput")
            cpar = dram.tile([128, 8], F32, kind="ExternalInput")
            outp = dram.tile([NIMG, 64, PIX], F16, kind="ExternalOutput")
            names.update(xta=xta.name, xsh=xsh.name, xin=xin.name,
                         w1a=w1a.name, w1bp=w1bp.name, w1bs=w1bs.name,
                         w2a8=w2a8.name, cpar=cpar.name, outp=outp.name)

            with tc.tile_pool(name="dramw", bufs=1, space="DRAM") as dramw, \
                 tc.tile_pool(name="wsb", bufs=1) as wsb, \
                 tc.tile_pool(name="ys", bufs=8) as yspool, \
                 tc.tile_pool(name="plane", bufs=3) as plpool, \
                 tc.tile_pool(name="planeh", bufs=2) as phpool, \
                 tc.tile_pool(name="hfp", bufs=2) as hf, \
                 tc.tile_pool(name="tiny", bufs=30) as tiny, \
                 tc.tile_pool(name="ps", bufs=7, space="PSUM") as ps:

                # ---- static parameter loads (w1a first: needed by matmul 0;
                # the rest issue on other queues / after first planes)
                w1as = wsb.tile([128, 9 * 64], F16, tag="w1a")
                nc.sync.dma_start(w1as[:], w1a[:])
                w1bps = wsb.tile([128, 3 * 64], F16, tag="w1bp")
                nc.scalar.dma_start(w1bps[:], w1bp[:])
                w1bss = wsb.tile([128, 3 * 64], F16, tag="w1bs")
                nc.scalar.dma_start(w1bss[:], w1bs[:])
                w2a8s = wsb.tile([128, 2 * 9 * 64], F8, tag="w2a8")
                nc.scalar.dma_start(w2a8s[:], w2a8[:])
                w2a8r = w2a8s.rearrange("p (two f) -> p two f", two=2)
                cpars = wsb.tile([128, 8], F32, tag="cpar")
                nc.scalar.dma_start(cpars[:], cpar[:])
                lhs_map = {"w1a": w1as, "w1bp": w1bps, "w1bs": w1bss}
                sums1 = wsb.tile([128, 56], F32, tag="sums1")
                sums1q = wsb.tile([128, 56], F32, tag="sums1q")
                # conv2 sums: img-A cols 0:56, img-B cols 56:112, all in
                # partitions 0:64 (DoubleRow psum lives at partitions 0:64);
                # partitions 64:128 zeroed so the stats shuffle-add is a no-op
                sums2 = wsb.tile([128, 112], F32, tag="sums2")
                sums2q = wsb.tile([128, 56], F32, tag="sums2q")
                nc.vector.memset(sums2[64:128, :], 0.0)
                if PHASES < 2:
                    nc.vector.memset(sums2[0:64, :], 0.0)
                    nc.vector.memset(sums2q[:], 0.0)

                def conv_img_pair(pcA, pcB, phA, phB, ops, dst_strip, sums_t,
                                  sumsq_t, pcol):
                    """One image pair -> 7 chunks in two waves (4+3); ops
                    outer within a wave so consecutive matmuls hit different
                    PSUM banks and weight loads amortize; wave evacuations
                    overlap the next wave's matmuls."""
                    plmap = {"pc": [pcA, pcB]}
                    if phA is not None:
                        plmap["ph"] = [phA, phB]
                    for k in plmap:
                        plmap[k] = [p.rearrange("p (r w) -> p r w", w=HP)
                                    for p in plmap[k]]
                    nops = len(ops)
                    for wave in (range(0, 4), range(4, 7)):
                        pts = {}
                        for cth in wave:
                            pts[cth] = ps.tile([128, CHW], F32, tag="ps",
                                               bufs=7, name=f"psum{cth}")
                        for oi, (pk, ln, co, di, dj) in enumerate(ops):
                            lt = lhs_map[ln]
                            for cth in wave:
                                r0 = 8 * cth + di
                                for j in range(2):
                                    plr = plmap[pk][j]
                                    rhs = plr[:, r0:r0 + 8, dj:dj + W]
                                    out = pts[cth][64 * j:64 * (j + 1), :] \
                                        .rearrange("p (r w) -> p r w", r=8)
                                    nc.tensor.matmul(
                                        out, lt[:, co:co + 64], rhs,
                                        start=(oi == 0), stop=(oi == nops - 1),
                                        tile_position=(0, 64 * j),
                                        skip_group_check=True)
                        for cth in wave:
                            nc.scalar.activation(
                                dst_strip[:, CHW * cth:CHW * (cth + 1)],
                                pts[cth][:], AF.Copy,
                                accum_out=sums_t[:, pcol * 7 + cth:pcol * 7 + cth + 1])
                            jk = ps.tile([128, CHW], F32, tag="psjk", bufs=1,
                                         name="psjk")
                            sl = dst_strip[:, CHW * cth:CHW * (cth + 1)]
                            nc.vector.scalar_tensor_tensor(
                                jk[:], sl, 1.0, sl, AO.bypass, AO.mult,
                                accum_out=sumsq_t[:, pcol * 7 + cth:pcol * 7 + cth + 1])

                def conv2_pair_fp8(pl8A, pl8B, dst_strip, pcol):
                    """conv2 via fp8 DoubleRow: per tap one matmul computes
                    all four E4M3 weight terms (pairs interleaved on the 2-dim,
                    term pair 3/4 pre-scaled 2^9 against spike copies *2^-9 on
                    partitions 64:128). DoubleRow psum must live at partitions
                    0:64, so each image gets its own psum tile; image B is
                    evacuated via a partition-moving DMA."""
                    plr8 = [p.rearrange("p (r w) -> p r w", w=HP)
                            for p in (pl8A, pl8B)]
                    for wave in (range(0, 3), range(3, 5), range(5, 7)):
                        pts = {}
                        for cth in wave:
                            for j in range(2):
                                pts[(cth, j)] = ps.tile(
                                    [128, CHW], F32, tag="ps", bufs=7,
                                    name=f"ps8_{cth}_{j}")
                        for a in range(9):
                            di, dj = a // 3, a % 3
                            lhs = w2a8r[:, :, a * 64:(a + 1) * 64]
                            for cth in wave:
                                r0 = 8 * cth + di
                                for j in range(2):
                                    rhs = plr8[j][:, r0:r0 + 8, dj:dj + W] \
                                        .unsqueeze(1) \
                                        .broadcast_to([128, 2, 8, W])
                                    out = pts[(cth, j)][0:64, :] \
                                        .rearrange("p (r w) -> p r w", r=8)
                                    nc.tensor.matmul(
                                        out, lhs, rhs,
                                        start=(a == 0), stop=(a == 8),
                                        perf_mode=PM.DoubleRow,
                                        tile_position=(0, 0),
                                        skip_group_check=True)
                        for cth in wave:
                            ccol = slice(CHW * cth, CHW * (cth + 1))
                            nc.scalar.activation(
                                dst_strip[0:64, ccol], pts[(cth, 0)][0:64, :],
                                AF.Copy,
                                accum_out=sums2[0:64, pcol * 7 + cth:pcol * 7 + cth + 1])
                            tmpb = hf.tile([128, CHW], F32, tag="tmpb",
                                           bufs=2)
                            nc.scalar.activation(
                                tmpb[0:64, :], pts[(cth, 1)][0:64, :],
                                AF.Copy,
                                accum_out=sums2[0:64, 56 + pcol * 7 + cth:56 + pcol * 7 + cth + 1])
                            nc.sync.dma_start(dst_strip[64:128, ccol],
                                              tmpb[0:64, :])
                            sl = dst_strip[:, ccol]
                            jk = ps.tile([128, CHW], F32, tag="psjk", bufs=1,
                                         name="psjk")
                            nc.vector.scalar_tensor_tensor(
                                jk[:], sl, 1.0, sl, AO.bypass, AO.mult,
                                accum_out=sums2q[:, pcol * 7 + cth:pcol * 7 + cth + 1])

                # ================= phase A: conv1 =================
                y1s = []
                for p in range(NPAIR):
                    tt_, bp = p // 2, p % 2
                    iA = tt_ * 4 + bp * 2
                    pcs, phs = [], []
                    for j in range(2):
                        i = iA + j
                        pc = plpool.tile([128, PP], F16, tag="ta", bufs=2)
                        nc.sync.dma_start(
                            pc[:], xta[i].rearrange("a b q -> (a b) q"))
                        pcs.append(pc)
                        ph = phpool.tile([128, PP], F16, tag="tb")
                        nc.gpsimd.dma_start(ph[0:64, :], xta[i, 0])
                        nc.gpsimd.dma_start(ph[64:128, :], xsh[i])
                        phs.append(ph)
                    strip = yspool.tile([128, PIX], F32, tag="ys")
                    y1s.append(strip)
                    conv_img_pair(pcs[0], pcs[1], phs[0], phs[1], ops1, strip,
                                  sums1, sums1q, p)

                # PE p-state warmup: dummy matmuls fill the stats1 barrier so
                # the clock gate stays at full speed when conv2 starts
                jkw = ps.tile([128, CHW], F32, tag="psjk", bufs=1,
                              name="psjk")
                for _ in range(60):
                    nc.tensor.matmul(jkw[0:64, :], w1as[:, 0:64],
                                     w1as[:, 0:CHW], start=True, stop=True,
                                     tile_position=(0, 0),
                                     skip_group_check=True)

                # ---- stats1 allreduce
                cc1i = dramw.tile([128, 2], F32)
                cc1o = dramw.tile([128, 2], F32, addr_space="Shared")
                acc1 = tiny.tile([128, 2], F32, tag="acc")
                nc.vector.tensor_reduce(acc1[:, 0:1], sums1[:], AX.X, AO.add)
                nc.vector.tensor_reduce(acc1[:, 1:2], sums1q[:], AX.X, AO.add)
                nc.sync.dma_start(cc1i[:], acc1[:])
                if NO_CC:
                    nc.sync.dma_start(cc1o[:], cc1i[:])
                else:
                    nc.gpsimd.collective_compute(
                        "AllReduce", AO.add, ins=[cc1i[:]], outs=[cc1o[:]],
                        replica_groups=[list(range(NCORES))])
                g1 = tiny.tile([128, 2], F32, tag="acc")
                nc.sync.dma_start(g1[:], cc1o[:])

                def stats_block(g, gamma, beta, rga, rgam, alpha):
                    gr = tiny.tile([128, 2], F32, tag="acc")
                    nc.sync.dma_start(gr[0:64, :], g[64:128, :])
                    nc.sync.dma_start(gr[64:128, :], g[0:64, :])
                    tot = tiny.tile([128, 2], F32, tag="acc")
                    nc.vector.tensor_tensor(tot[:], g[:], gr[:], AO.add)
                    mean = tiny.tile([128, 1], F32, tag="t1")
                    nc.vector.tensor_scalar(mean[:], tot[:, 0:1], 1.0 / NG,
                                            None, AO.mult)
                    msq = tiny.tile([128, 1], F32, tag="t1")
                    nc.vector.tensor_scalar(msq[:], tot[:, 1:2], 1.0 / NG,
                                            None, AO.mult)
                    m2 = tiny.tile([128, 1], F32, tag="t1")
                    nc.vector.scalar_tensor_tensor(m2[:], mean[:], 1.0, mean[:],
                                                   AO.bypass, AO.mult)
                    var = tiny.tile([128, 1], F32, tag="t1")
                    nc.vector.tensor_tensor(var[:], msq[:], m2[:], AO.subtract)
                    epst = tiny.tile([128, 1], F32, tag="t1")
                    nc.vector.memset(epst[:], EPS)
                    std = tiny.tile([128, 1], F32, tag="t1")
                    nc.scalar.activation(std[:], var[:], AF.Sqrt, bias=epst[:])
                    rstd = tiny.tile([128, 1], F32, tag="t1")
                    nc.vector.reciprocal(rstd[:], std[:])
                    sc = tiny.tile([128, 1], F32, tag="t1")
                    nc.vector.tensor_tensor(sc[:], gamma, rstd[:], AO.mult)
                    nmsc = tiny.tile([128, 1], F32, tag="t1")
                    nc.vector.scalar_tensor_tensor(nmsc[:], mean[:], -1.0, sc[:],
                                                   AO.mult, AO.mult)
                    bi = tiny.tile([128, 1], F32, tag="t1")
                    nc.vector.tensor_tensor(bi[:], beta, nmsc[:], AO.add)
                    stdrg = tiny.tile([128, 1], F32, tag="t1")
                    nc.vector.tensor_tensor(stdrg[:], std[:], rga, AO.mult)
                    nbst = tiny.tile([128, 1], F32, tag="t1")
                    nc.vector.scalar_tensor_tensor(nbst[:], bi[:], -alpha,
                                                   stdrg[:], AO.mult, AO.mult)
                    th = tiny.tile([128, 1], F32, tag="t1")
                    nc.vector.tensor_tensor(th[:], stdrg[:], nbst[:], AO.add)
                    bstd = tiny.tile([128, 1], F32, tag="t1")
                    nc.vector.tensor_tensor(bstd[:], bi[:], std[:], AO.mult)
                    gamv = tiny.tile([128, 1], F32, tag="t1")
                    nc.vector.tensor_tensor(gamv[:], bstd[:], rgam, AO.mult)
                    rscv = tiny.tile([128, 1], F32, tag="t1")
                    nc.vector.tensor_tensor(rscv[:], std[:], rgam, AO.mult)
                    gmw = tiny.tile([128, 1], F32, tag="t1")
                    nc.vector.tensor_scalar(gmw[:], gamv[:], 1.0 - alpha, None,
                                            AO.mult)
                    return th, gamv, rscv, gmw

                th1, gm1, _rsc1, gmw1 = stats_block(
                    g1, cpars[:, 0:1], cpars[:, 1:2], cpars[:, 4:5],
                    cpars[:, 6:7], alpha1)

                # ============ phase B + C: LIF1 + conv2 ============
                # spikes + plane DMAs first (conv2 start latency), P-updates
                # after; wv on Act via gmw1; spikes in fp8 with a *2^-9 copy
                # on partitions 64:128 feeding DoubleRow term pair 3/4.
                y2s = [None] * NPAIR
                ta8_gen = [0]
                for bp in range(2 if PHASES >= 2 else 0):
                    Pprev = [None] * NQ
                    for t in range(1, 5):
                        p = (t - 1) * 2 + bp
                        tas_pair = []
                        for j in range(2):
                            tas = plpool.tile([128, PP], F8, tag="ta8", bufs=4)
                            if ta8_gen[0] < 4:
                                # fresh buffer: zero once; later generations
                                # keep zero borders (DMAs write interior only)
                                nc.gpsimd.memset(tas[:], 0.0)
                            ta8_gen[0] += 1
                            tas_pair.append(tas)
                        qas = []
                        for half in range(2):
                            # spikes for one half-image (2 quarters) into one
                            # f8 tile, then 4 plane DMAs per half: wave0 of
                            # conv2 needs only half 0, so matmuls start early
                            s8 = hf.tile([128, 2 * QL], F8, tag="s8", bufs=2)
                            s8s = hf.tile([128, 2 * QL], F8, tag="s8s",
                                          bufs=2)
                            for hh in range(2):
                                hq = half * 2 + hh
                                off = QL * hq
                                ysl = y1s[p][:, off:off + QL]
                                if t == 1:
                                    qa = ysl
                                else:
                                    q = hf.tile([128, QL], F32, tag="q2",
                                                bufs=4)
                                    nc.gpsimd.tensor_tensor(q[:], ysl,
                                                            Pprev[hq][:],
                                                            AO.add)
                                    qa = q[:]
                                qas.append(qa)
                                so = slice(QL * hh, QL * (hh + 1))
                                nc.vector.tensor_scalar(s8[:, so], qa, th1[:],
                                                        None, AO.is_ge)
                                nc.vector.tensor_scalar(s8s[:, so], qa,
                                                        th1[:], 2.0 ** -9,
                                                        AO.is_ge, AO.mult)
                            for j in range(2):
                                tasr = tas_pair[j].rearrange(
                                    "p (r w) -> p r w", w=HP)
                                dsti = tasr[:, 1 + 28 * half:1 + 28 * (half + 1),
                                            1:1 + W]
                                srcs = s8[64 * j:64 * (j + 1), :] \
                                    .rearrange("p (r w) -> p r w", w=W)
                                srcss = s8s[64 * j:64 * (j + 1), :] \
                                    .rearrange("p (r w) -> p r w", w=W)
                                nc.sync.dma_start(dsti[0:64], srcs)
                                nc.sync.dma_start(dsti[64:128], srcss)
                        if t < 4:
                            for hq in range(NQ):
                                qa = qas[hq]
                                wv = hf.tile([128, QL], F32, tag="wv", bufs=2)
                                nc.scalar.activation(wv[:], qa, AF.Identity,
                                                     bias=gmw1[:],
                                                     scale=1.0 - alpha1)
                                Pn = hf.tile([128, QL], F32, tag="pp",
                                             bufs=8)
                                nc.vector.scalar_tensor_tensor(
                                    Pn[:], qa, th1[:], wv[:], AO.is_lt,
                                    AO.mult)
                                Pprev[hq] = Pn
                        strip2 = yspool.tile([128, PIX], F32, tag="ys")
                        y2s[p] = strip2
                        conv2_pair_fp8(tas_pair[0], tas_pair[1], strip2, p)

                # ---- stats2 allreduce
                cc2i = dramw.tile([128, 2], F32)
                cc2o = dramw.tile([128, 2], F32, addr_space="Shared")
                acc2 = tiny.tile([128, 2], F32, tag="acc")
                nc.vector.tensor_reduce(acc2[:, 0:1], sums2[:], AX.X, AO.add)
                nc.vector.tensor_reduce(acc2[:, 1:2], sums2q[:], AX.X, AO.add)
                nc.sync.dma_start(cc2i[:], acc2[:])
                if NO_CC:
                    nc.sync.dma_start(cc2o[:], cc2i[:])
                else:
                    nc.gpsimd.collective_compute(
                        "AllReduce", AO.add, ins=[cc2i[:]], outs=[cc2o[:]],
                        replica_groups=[list(range(NCORES))])
                g2 = tiny.tile([128, 2], F32, tag="acc")
                nc.sync.dma_start(g2[:], cc2o[:])
                th2, gm2, rsc2, gmw2 = stats_block(
                    g2, cpars[:, 2:3], cpars[:, 3:4], cpars[:, 5:6],
                    cpars[:, 7:8], alpha2)
                nth2 = tiny.tile([128, 1], F32, tag="t1")
                nc.vector.tensor_scalar(nth2[:], th2[:], -1.0, None, AO.mult)

                # ============ phase D: residual + LIF2 ============
                # spike = Sign(q2 - th2) in fp16; host maps sign>=0 -> 1.
                # t-major so the 8 (bp,hq-pair) chains interleave; z on
                # GpSimd, q2 fused stt on DVE, spike on Act, out via Act DGE.
                Pprev2 = {}
                pend_pn = []
                pend_out = []
                for t in range(1 if PHASES >= 3 else 5, 5):
                    for bp in range(2):
                        p = (t - 1) * 2 + bp
                        iA = (t - 1) * 4 + bp * 2
                        for hq in range(NQ):
                            off = QL * hq
                            xs = hf.tile([128, QL], F32, tag="xs", bufs=2)
                            nc.sync.dma_start(
                                xs[:], xin[iA:iA + 2, :, off:off + QL]
                                .rearrange("a b q -> (a b) q"))
                            if t == 1:
                                zv = y2s[p][:, off:off + QL]
                            else:
                                z = hf.tile([128, QL], F32, tag="q2", bufs=4)
                                nc.gpsimd.tensor_tensor(
                                    z[:], y2s[p][:, off:off + QL],
                                    Pprev2[(bp, hq)][:], AO.add)
                                zv = z[:]
                            q2 = hf.tile([128, QL], F32, tag="q2", bufs=4)
                            nc.vector.scalar_tensor_tensor(
                                q2[:], xs[:], rsc2[:], zv, AO.mult, AO.add)
                            q2v = q2[:]
                            ot = hf.tile([128, QL], F16, tag="s1t", bufs=2)
                            nc.scalar.activation(ot[:], q2v, AF.Sign,
                                                 bias=nth2[:])
                            # defer the out-DMA by one chain so its wait on ot
                            # never blocks the Pool SEQ ahead of z-adds
                            pend_out.append((iA, off, ot))
                            if len(pend_out) > 1:
                                iAp, offp, otp = pend_out.pop(0)
                                nc.scalar.dma_start(
                                    outp[iAp:iAp + 2, :, offp:offp + QL]
                                    .rearrange("a b q -> (a b) q"), otp[:])
                            if t < 4:
                                wv2 = hf.tile([128, QL], F32, tag="wv",
                                              bufs=2)
                                nc.scalar.activation(wv2[:], q2v, AF.Identity,
                                                     bias=gmw2[:],
                                                     scale=1.0 - alpha2)
                                # defer Pn by one chain so the DVE queue head
                                # never waits on this chain's wv2 (Act)
                                pend_pn.append((bp, hq, wv2, q2))
                                if len(pend_pn) > 1:
                                    bpp, hqp, wvp, q2p = pend_pn.pop(0)
                                    Pn = hf.tile([128, QL], F32, tag="pp",
                                                 bufs=8)
                                    nc.vector.scalar_tensor_tensor(
                                        Pn[:], q2p[:], th2[:], wvp[:],
                                        AO.is_lt, AO.mult)
                                    Pprev2[(bpp, hqp)] = Pn
                    while pend_pn:
                        bpp, hqp, wvp, q2p = pend_pn.pop(0)
                        Pn = hf.tile([128, QL], F32, tag="pp", bufs=8)
                        nc.vector.scalar_tensor_tensor(
                            Pn[:], q2p[:], th2[:], wvp[:], AO.is_lt, AO.mult)
                        Pprev2[(bpp, hqp)] = Pn
                while pend_out:
                    iAp, offp, otp = pend_out.pop(0)
                    nc.scalar.dma_start(
                        outp[iAp:iAp + 2, :, offp:offp + QL]
                        .rearrange("a b q -> (a b) q"), otp[:])

    nc.compile()
    return nc, names


def _sigmoid(x):
    return 1.0 / (1.0 + np.exp(-float(x)))


def prepare(x, conv1_w, bn1_gamma, bn1_beta, lif1_w, conv2_w, bn2_gamma,
            bn2_beta, lif2_w):
    x = np.ascontiguousarray(np.asarray(x, np.float32))
    conv1_w = np.asarray(conv1_w, np.float32)
    conv2_w = np.asarray(conv2_w, np.float32)

    a1 = _sigmoid(np.asarray(lif1_w).reshape(-1)[0])
    a2 = _sigmoid(np.asarray(lif2_w).reshape(-1)[0])

    key = (round(a1, 12), round(a2, 12))
    if key not in _prog_cache:
        _prog_cache[key] = _build(a1, a2)
    nc, names = _prog_cache[key]

    # fp16 hi/lo split of x, padded planes (encoding only; exact split)
    xh = x.astype(np.float16)
    xl = (x - xh.astype(np.float32)).astype(np.float16)
    xpad = np.zeros((T, B, C, 2, HP, HP), np.float16)
    xpad[:, :, :, 0, 1:57, 1:57] = xh
    xpad[:, :, :, 1, 1:57, 1:57] = xl
    xpad = np.ascontiguousarray(xpad.transpose(0, 1, 3, 2, 4, 5))  # t,b,2,c,hp,hp
    # xhi shifted one column left (tap dj=1 via partitions 64:128)
    xshp = np.zeros((T, B, C, HP, HP), np.float16)
    xshp[:, :, :, 1:57, 0:56] = xh

    import ml_dtypes
    F8NP = ml_dtypes.float8_e4m3

    w1h = conv1_w.astype(np.float16)
    w1l = (conv1_w - w1h.astype(np.float32)).astype(np.float16)

    def tapstack(wtop, wbot):
        out = np.zeros((128, 9 * 64), np.float16)
        for a in range(9):
            di, dj = a // 3, a % 3
            out[0:64, a * 64:(a + 1) * 64] = wtop[:, :, di, dj].T
            out[64:128, a * 64:(a + 1) * 64] = wbot[:, :, di, dj].T
        return out

    w1a_np = tapstack(w1h, w1h)
    w1bp_np = np.zeros((128, 3 * 64), np.float16)
    w1bs_np = np.zeros((128, 3 * 64), np.float16)
    for di in range(3):
        w1bp_np[0:64, di * 64:(di + 1) * 64] = w1l[:, :, di, 0].T
        w1bp_np[64:128, di * 64:(di + 1) * 64] = w1l[:, :, di, 1].T
        w1bs_np[0:64, di * 64:(di + 1) * 64] = w1l[:, :, di, 2].T

    # conv2 weights: 4-term greedy E4M3 decomposition; terms 3/4 stored
    # scaled by 2^9 (device spikes *2^-9 on partitions 64:128 compensate)
    w2d = conv2_w.astype(np.float64)
    t1 = w2d.astype(F8NP)
    r = w2d - t1.astype(np.float64)
    t2 = r.astype(F8NP)
    r = r - t2.astype(np.float64)
    t3 = (r * 512.0).astype(F8NP)
    r = r - t3.astype(np.float64) / 512.0
    t4 = (r * 512.0).astype(F8NP)
    w2a8_np = np.zeros((128, 2, 9 * 64), F8NP)
    for a in range(9):
        di, dj = a // 3, a % 3
        w2a8_np[0:64, 0, a * 64:(a + 1) * 64] = t1[:, :, di, dj].T
        w2a8_np[0:64, 1, a * 64:(a + 1) * 64] = t2[:, :, di, dj].T
        w2a8_np[64:128, 0, a * 64:(a + 1) * 64] = t3[:, :, di, dj].T
        w2a8_np[64:128, 1, a * 64:(a + 1) * 64] = t4[:, :, di, dj].T
    w2a8_np = np.ascontiguousarray(w2a8_np.reshape(128, 2 * 9 * 64))

    def dup(v):
        v = np.asarray(v, np.float32).reshape(64)
        return np.concatenate([v, v])

    cpar_np = np.zeros((128, 8), np.float32)
    cpar_np[:, 0] = dup(bn1_gamma)
    cpar_np[:, 1] = dup(bn1_beta)
    cpar_np[:, 2] = dup(bn2_gamma)
    cpar_np[:, 3] = dup(bn2_beta)
    cpar_np[:, 4] = 1.0 / (a1 * dup(bn1_gamma))
    cpar_np[:, 5] = 1.0 / (a2 * dup(bn2_gamma))
    cpar_np[:, 6] = 1.0 / dup(bn1_gamma)
    cpar_np[:, 7] = 1.0 / dup(bn2_gamma)

    in_maps = []
    for k in range(NCORES):
        xta_np = np.ascontiguousarray(
            xpad[:, 4 * k:4 * k + 4].reshape(NIMG, 2, 64, PP))
        xsh_np = np.ascontiguousarray(
            xshp[:, 4 * k:4 * k + 4].reshape(NIMG, 64, PP))
        xin_np = np.ascontiguousarray(
            x[:, 4 * k:4 * k + 4].reshape(NIMG, 64, PIX))
        in_maps.append({
            names['xta']: xta_np,
            names['xsh']: xsh_np,
            names['xin']: xin_np,
            names['w1a']: w1a_np,
            names['w1bp']: w1bp_np,
            names['w1bs']: w1bs_np,
            names['w2a8']: w2a8_np,
            names['cpar']: cpar_np,
        })

    return nc, names, in_maps


def kernel(**inputs):
    from concourse.bass_utils import run_bass_kernel_spmd
    nc, names, in_maps = prepare(**inputs)
    res = run_bass_kernel_spmd(nc, in_maps, core_ids=list(range(NCORES)))
    global LAST_RES, LAST_NAMES
    LAST_RES, LAST_NAMES = res, names
    out = np.empty((T, B, C, H, W), np.float32)
    for k in range(NCORES):
        o = res.results[k][names['outp']]
        s = (o.astype(np.float32) >= 0.0).astype(np.float32)
        out[:, 4 * k:4 * k + 4] = s.reshape(T, BL, C, H, W)
    return out


if __name__ == "__main__":
    rng = np.random.default_rng(0)
    xs = rng.standard_normal((T, B, C, H, W)).astype(np.float32)
    w1 = (rng.standard_normal((64, 64, 3, 3)) * 0.05).astype(np.float32)
    w2 = (rng.standard_normal((64, 64, 3, 3)) * 0.05).astype(np.float32)
    o = kernel(x=xs, conv1_w=w1, bn1_gamma=np.ones(64, np.float32),
               bn1_beta=np.zeros(64, np.float32),
               lif1_w=np.zeros(1, np.float32), conv2_w=w2,
               bn2_gamma=np.ones(64, np.float32),
               bn2_beta=np.zeros(64, np.float32),
               lif2_w=np.zeros(1, np.float32))
    print("ran:", o.shape, float(o.mean()))
